# revision 8
# baseline (speedup 1.0000x reference)
"""Trainium2 Bass kernel for nn_DeformationNetworkSeparate.

Data-parallel over 8 NeuronCores: each core gets N/8 points; the ~0.85M-param
MLP weights are replicated (fp16 on-chip, fp32 PSUM accumulation).

Per-core pipeline (feature-major activations [features, points]):
  posenc: ang = freq*x computed exactly as the fp32 reference (single fp32
  multiply), range-reduced with a 3-term Cody-Waite cascade + magic-number
  round, sin/cos via the ACT Sin spline on [-pi, pi] (cos = sin wrapped +pi/2).
  MLP: chained fp16 matmuls (K on partitions), ReLU+bias epilogues split
  between ScalarE and VectorE, final outputs transposed back to point-major
  via identity matmuls, quaternion normalize with Newton rsqrt.
"""
import math
import numpy as np

import concourse.bass as bass
import concourse.mybir as mybir
import concourse.tile as tile
from concourse import bacc
from concourse.bass_utils import run_bass_kernel_spmd

dt = mybir.dt
AL = mybir.AluOpType
AF = mybir.ActivationFunctionType

N_CORES = 8
N_TOTAL = 262144
S = N_TOTAL // N_CORES          # 32768 points per core
T = 1024                        # points per tile
NT = S // T                     # 32 tiles
COORD_L, QUAT_L = 10, 15

# Row layout of the reduction tile R [128, T]:
#   0:30   x-branch angles (k major, d minor), 30:32 zero pad
#   32:64  unused
#   64:124 q-branch angles, 124:128 zero pad
XR0, XNR = 0, 30
QR0, QNR = 64, 60

TWO_PI = 2.0 * math.pi
MAGIC = float(np.float32(1.5 * 2 ** 23))
C1 = np.float32(6.28125)
_m, _e = np.frexp(np.float64(TWO_PI) - np.float64(C1))
C2 = np.float32(np.ldexp(np.round(_m * 2 ** 12) / 2 ** 12, _e))
C3 = np.float32(np.float64(TWO_PI) - np.float64(C1) - np.float64(C2))

# layer dims after the t-column fold (K1 includes pad rows)
DIMS_X = [(64, 256), (256, 256), (256, 256), (256, 256), (256, 256),
          (256, 256), (256, 128), (128, 128), (128, 3)]
DIMS_Q = [(128, 256), (256, 256), (256, 256), (256, 256), (256, 256),
          (256, 256), (256, 256), (256, 128), (128, 4)]

# rough per-op costs (ns) for ACT/DVE load balancing of epilogues
EPI_ACT, EPI_DVE = 1000.0, 1190.0

_BUILT = {}


def _emit_branch(nc, pools, tag, dims, h1_tile, wt_tiles, bt_tiles, out_tile):
    """Emit matmul+epilogue chain for one branch. h1_tile: [K1, T] fp16.
    out_tile: fp32 [dout_last, T]. Returns nothing."""
    sb, psum_mm, balance = pools
    cur = h1_tile
    cur_kt = 1  # h1 is a single K tile (K1 <= 128)
    nlayers = len(dims)
    for li, (din, dout) in enumerate(dims):
        relu = li < nlayers - 1
        ktiles = (din + 127) // 128
        mtiles = (dout + 127) // 128
        mdim = min(dout, 128)
        if relu:
            nxt = sb.tile([128, mtiles, T], dt.float16, tag=f"{tag}h{mtiles}")
        for m in range(mtiles):
            ps = psum_mm.tile([128, T], dt.float32, tag="mm")
            for fh in range(T // 512):
                fs = slice(fh * 512, (fh + 1) * 512)
                for k in range(ktiles):
                    if li == 0:
                        rhs = cur[:, fs]
                    else:
                        rhs = cur[:, k, fs]
                    if ktiles == 1:
                        lhsT = wt_tiles[li][:, m * 128:m * 128 + mdim]
                    else:
                        lhsT = wt_tiles[li][:, k, m * 128:m * 128 + mdim]
                    nc.tensor.matmul(ps[:mdim, fs], lhsT, rhs,
                                     start=(k == 0), stop=(k == ktiles - 1))
            b_ap = bt_tiles[li][:mdim, m:m + 1]
            if relu:
                out_ap = nxt[:, m, :]
            else:
                out_ap = out_tile[:mdim, :]
            # pick engine greedily by accumulated load
            if balance["act"] + EPI_ACT <= balance["dve"] + EPI_DVE:
                balance["act"] += EPI_ACT
                nc.scalar.activation(out_ap, ps[:mdim, :],
                                     AF.Relu if relu else AF.Identity,
                                     bias=b_ap, scale=1.0)
            else:
                balance["dve"] += EPI_DVE
                if relu:
                    nc.vector.tensor_scalar(out_ap, ps[:mdim, :], b_ap, 0.0,
                                            AL.add, AL.max)
                else:
                    nc.vector.tensor_scalar(out_ap, ps[:mdim, :], b_ap, None,
                                            AL.add)
        if relu:
            cur = nxt
            cur_kt = mtiles


def _build_program(nt=NT, dump=False):
    nc = bacc.Bacc("TRN2", target_bir_lowering=False, debug=False)

    repx_d = nc.dram_tensor("repx", [32, S], dt.float32, kind="ExternalInput")
    repq_d = nc.dram_tensor("repq", [64, S], dt.float32, kind="ExternalInput")
    s_d = nc.dram_tensor("sfreq", [128, 1], dt.float32, kind="ExternalInput")
    s2_d = nc.dram_tensor("sfreq2", [128, 1], dt.float32, kind="ExternalInput")
    i3_d = nc.dram_tensor("ident3", [3, 3], dt.float32, kind="ExternalInput")
    i4_d = nc.dram_tensor("ident4", [4, 4], dt.float32, kind="ExternalInput")

    w_d, b_d = {}, {}
    for tag, dims in (("x", DIMS_X), ("q", DIMS_Q)):
        for li, (din, dout) in enumerate(dims):
            kt = (din + 127) // 128
            if kt == 1:
                wshape = [din, dout]
            else:
                wshape = [128, kt, dout]
            w_d[tag, li] = nc.dram_tensor(f"w{tag}{li}", wshape, dt.float16,
                                          kind="ExternalInput")
            b_d[tag, li] = nc.dram_tensor(f"b{tag}{li}",
                                          [min(dout, 128), (dout + 127) // 128],
                                          dt.float32, kind="ExternalInput")

    ox_d = nc.dram_tensor("out_x", [S, 3], dt.float32, kind="ExternalOutput")
    oq_d = nc.dram_tensor("out_q", [S, 4], dt.float32, kind="ExternalOutput")

    dump_d = {}
    if dump:
        for name, shape, ddt in [("d_hx1", [64, T], dt.float16),
                                 ("d_hq1", [128, T], dt.float16),
                                 ("d_r", [128, T], dt.float32),
                                 ("d_ox", [3, T], dt.float32),
                                 ("d_oq", [4, T], dt.float32),
                                 ("d_sq", [128, 8, 4], dt.float32)]:
            dump_d[name] = nc.dram_tensor(name, shape, ddt, kind="ExternalOutput")

    with tile.TileContext(nc) as tc:
        with tc.tile_pool(name="wpool", bufs=1) as wp, \
             tc.tile_pool(name="sb", bufs=2) as sb, \
             tc.tile_pool(name="pmm", bufs=3, space="PSUM") as psum_mm, \
             tc.tile_pool(name="ptr", bufs=2, space="PSUM") as psum_tr:

            # ---- persistent tiles: weights, biases, consts ----
            wt, bt = {}, {}
            for tag, dims in (("x", DIMS_X), ("q", DIMS_Q)):
                for li, (din, dout) in enumerate(dims):
                    kt = (din + 127) // 128
                    shape = [din, dout] if kt == 1 else [128, kt, dout]
                    w = wp.tile(shape, dt.float16, tag=f"w{tag}{li}")
                    nc.sync.dma_start(w[:], w_d[tag, li][:])
                    wt[tag, li] = w
                    b = wp.tile([min(dout, 128), (dout + 127) // 128],
                                dt.float32, tag=f"b{tag}{li}")
                    nc.sync.dma_start(b[:], b_d[tag, li][:])
                    bt[tag, li] = b
            s_t = wp.tile([128, 1], dt.float32, tag="sfreq")
            nc.sync.dma_start(s_t[:], s_d[:])
            s2_t = wp.tile([128, 1], dt.float32, tag="sfreq2")
            nc.sync.dma_start(s2_t[:], s2_d[:])
            i3_t = wp.tile([3, 3], dt.float32, tag="i3")
            nc.sync.dma_start(i3_t[:], i3_d[:])
            i4_t = wp.tile([4, 4], dt.float32, tag="i4")
            nc.sync.dma_start(i4_t[:], i4_d[:])
            magic_t = wp.tile([128, 8], dt.int32, tag="magic")
            nc.vector.memset(magic_t[:], 0x5F3759DF)

            balance = {"act": 0.0, "dve": 0.0}
            for t_i in range(nt):
                cs = slice(t_i * T, (t_i + 1) * T)
                # ---- posenc ----
                R = sb.tile([128, T], dt.float32, tag="R")
                nc.sync.dma_start(R[0:32, :], repx_d[:, cs])
                nc.sync.dma_start(R[64:128, :], repq_d[:, cs])
                ang = sb.tile([128, T], dt.float32, tag="ang")
                nc.vector.tensor_scalar(ang[:], R[:], s_t[:, 0:1], None, AL.mult)
                u = sb.tile([128, T], dt.float32, tag="u")
                nc.vector.tensor_scalar(u[:], R[:], s2_t[:, 0:1], None, AL.mult)
                nf = sb.tile([128, T], dt.float32, tag="nf")
                nc.vector.tensor_scalar(nf[:], u[:], MAGIC, MAGIC, AL.add, AL.subtract)
                r = sb.tile([128, T], dt.float32, tag="r")
                nc.vector.cody_waite_cascade(r[:], ang[:], nf[:],
                                             float(C1), float(C2), float(C3))
                # ScalarE only reads/writes at partition base 0, and the custom
                # DVE ops cannot shift partition base — so: wrap full tiles at
                # base 0, then assemble the Sin inputs with plain tensor_scalar
                # copies (DVE handles arbitrary base shifts).
                rw0 = sb.tile([128, T], dt.float32, tag="rw0")
                nc.vector.add_range_wrap(rw0[:], r[:], 0.0, math.pi, TWO_PI)
                rwc = sb.tile([128, T], dt.float32, tag="rwc")
                nc.vector.add_range_wrap(rwc[:], r[:], math.pi / 2, math.pi, TWO_PI)
                rwx = sb.tile([64, T], dt.float32, tag="rwx")
                nc.vector.tensor_scalar(rwx[0:32, :], rw0[0:32, :], 1.0, None, AL.mult)
                nc.vector.tensor_scalar(rwx[32:64, :], rwc[0:32, :], 1.0, None, AL.mult)
                rwq = sb.tile([128, T], dt.float32, tag="rwq")
                nc.vector.tensor_scalar(rwq[0:64, :], rw0[64:128, :], 1.0, None, AL.mult)
                nc.vector.tensor_scalar(rwq[64:128, :], rwc[64:128, :], 1.0, None, AL.mult)
                hx1 = sb.tile([64, T], dt.float16, tag="hx1")
                nc.scalar.activation(hx1[:], rwx[:], AF.Sin)
                hq1 = sb.tile([128, T], dt.float16, tag="hq1")
                nc.scalar.activation(hq1[:], rwq[:], AF.Sin)
                balance["dve"] += 3 * 594 + 3 * 1224 + 4 * 594
                balance["act"] += 2 * 1040

                # ---- MLP branches ----
                ox = sb.tile([3, T], dt.float32, tag="ox")
                oq = sb.tile([4, T], dt.float32, tag="oq")
                pools = (sb, psum_mm, balance)
                _emit_branch(nc, pools, "x", DIMS_X, hx1,
                             {li: wt["x", li] for li in range(9)},
                             {li: bt["x", li] for li in range(9)}, ox)
                _emit_branch(nc, pools, "q", DIMS_Q, hq1,
                             {li: wt["q", li] for li in range(9)},
                             {li: bt["q", li] for li in range(9)}, oq)

                # ---- transpose to point-major via identity matmuls ----
                SX = sb.tile([128, 8, 3], dt.float32, tag="SX")
                SQ = sb.tile([128, 8, 4], dt.float32, tag="SQ")
                for ch in range(8):
                    chs = slice(ch * 128, (ch + 1) * 128)
                    px = psum_tr.tile([128, 4], dt.float32, tag="tr")
                    nc.tensor.matmul(px[:, 0:3], ox[:, chs], i3_t[:],
                                     start=True, stop=True)
                    nc.scalar.activation(SX[:, ch, :], px[:, 0:3], AF.Copy)
                    pq = psum_tr.tile([128, 4], dt.float32, tag="tr")
                    nc.tensor.matmul(pq[:, 0:4], oq[:, chs], i4_t[:],
                                     start=True, stop=True)
                    nc.scalar.activation(SQ[:, ch, :], pq[:, 0:4], AF.Copy)
                balance["act"] += 16 * 150

                # ---- quaternion normalize (Newton rsqrt) ----
                sq = sb.tile([128, 8, 4], dt.float32, tag="sq")
                nc.vector.tensor_tensor(sq[:], SQ[:], SQ[:], AL.mult)
                n2 = sb.tile([128, 8], dt.float32, tag="n2")
                nc.vector.reduce_sum(n2[:, :, None], sq[:], axis=mybir.AxisListType.X)
                sh = sb.tile([128, 8], dt.int32, tag="sh")
                nc.vector.tensor_scalar(sh[:], n2[:].bitcast(dt.int32), 1, None,
                                        AL.logical_shift_right)
                y0i = sb.tile([128, 8], dt.int32, tag="y0i")
                nc.vector.tensor_tensor(y0i[:], magic_t[:], sh[:], AL.subtract)
                ycur = y0i[:].bitcast(dt.float32)
                for it in range(3):
                    t1 = sb.tile([128, 8], dt.float32, tag=f"nt1_{it}")
                    t2 = sb.tile([128, 8], dt.float32, tag=f"nt2_{it}")
                    nc.vector.tensor_tensor(t1[:], ycur, ycur, AL.mult)
                    nc.vector.tensor_tensor(t2[:], t1[:], n2[:], AL.mult)
                    nc.vector.tensor_scalar(t1[:], t2[:], -0.5, 1.5, AL.mult, AL.add)
                    yt = sb.tile([128, 8], dt.float32, tag=f"ny_{it}")
                    nc.vector.tensor_tensor(yt[:], ycur, t1[:], AL.mult)
                    ycur = yt[:]
                qn = sb.tile([128, 8, 4], dt.float32, tag="qn")
                nc.vector.tensor_tensor(
                    qn[:], SQ[:], ycur[:, :, None].to_broadcast((128, 8, 4)),
                    AL.mult)
                balance["dve"] += 17 * 95

                # ---- outputs ----
                nc.sync.dma_start(
                    ox_d[cs, :].rearrange("(c p) d -> p c d", p=128), SX[:])
                nc.sync.dma_start(
                    oq_d[cs, :].rearrange("(c p) d -> p c d", p=128), qn[:])
                if dump and t_i == 0:
                    nc.sync.dma_start(dump_d["d_hx1"][:], hx1[:])
                    nc.sync.dma_start(dump_d["d_hq1"][:], hq1[:])
                    nc.sync.dma_start(dump_d["d_r"][:], r[:])
                    nc.sync.dma_start(dump_d["d_ox"][:], ox[:])
                    nc.sync.dma_start(dump_d["d_oq"][:], oq[:])
                    nc.sync.dma_start(dump_d["d_sq"][:], SQ[:])

    nc.compile()
    return nc


def _prep_host(x, q, t, params_x, params_q):
    """Build all per-core input arrays."""
    t_val = float(np.asarray(t).reshape(-1)[0])
    xT = np.ascontiguousarray(np.asarray(x, np.float32).T)   # [3, N]
    qT = np.ascontiguousarray(np.asarray(q, np.float32).T)   # [4, N]

    repx = np.zeros((32, N_TOTAL), np.float32)
    for r in range(30):
        repx[r] = xT[r % 3]
    repq = np.zeros((64, N_TOTAL), np.float32)
    for r in range(60):
        repq[r] = qT[r % 4]

    freqs_x = (2.0 ** np.arange(COORD_L, dtype=np.float32)) * np.float32(np.pi)
    freqs_q = (2.0 ** np.arange(QUAT_L, dtype=np.float32)) * np.float32(np.pi)
    s = np.zeros((128, 1), np.float32)
    for r in range(30):
        s[r, 0] = freqs_x[r // 3]
    for r in range(60):
        s[64 + r, 0] = freqs_q[r // 4]
    s2 = (s.astype(np.float64) / TWO_PI).astype(np.float32)

    def cvt_w(W):
        return np.asarray(W, np.float32)

    wmap, bmap = {}, {}
    # ---- first layers: permute posenc columns into our row layout, fold t ----
    W1x, b1x = params_x[0]
    W1x = cvt_w(W1x)
    w1xp = np.zeros((64, 256), np.float32)
    for r in range(30):
        k, d = r // 3, r % 3
        w1xp[r] = W1x[:, k * 6 + d]          # sin rows
        w1xp[32 + r] = W1x[:, k * 6 + 3 + d]  # cos rows
    wmap["x", 0] = w1xp.astype(np.float16)
    b1xf = (np.asarray(b1x, np.float64) + np.asarray(W1x[:, 60], np.float64) * t_val)
    bmap["x", 0] = b1xf.astype(np.float32)

    W1q, b1q = params_q[0]
    W1q = cvt_w(W1q)
    w1qp = np.zeros((128, 256), np.float32)
    for r in range(60):
        k, d = r // 4, r % 4
        w1qp[r] = W1q[:, k * 8 + d]
        w1qp[64 + r] = W1q[:, k * 8 + 4 + d]
    wmap["q", 0] = w1qp.astype(np.float16)
    b1qf = (np.asarray(b1q, np.float64) + np.asarray(W1q[:, 120], np.float64) * t_val)
    bmap["q", 0] = b1qf.astype(np.float32)

    for tag, params, dims in (("x", params_x, DIMS_X), ("q", params_q, DIMS_Q)):
        for li in range(1, 9):
            W, b = params[li]
            W = cvt_w(W)                      # [dout, din]
            WT = W.T                          # [din, dout]
            din, dout = dims[li]
            kt = (din + 127) // 128
            if kt == 1:
                wmap[tag, li] = WT.astype(np.float16)
            else:
                wmap[tag, li] = np.ascontiguousarray(
                    WT.reshape(kt, 128, dout).transpose(1, 0, 2)).astype(np.float16)
            bmap[tag, li] = np.asarray(b, np.float32)

    # bias arrays -> [min(dout,128), mtiles]
    barr = {}
    for (tag, li), b in bmap.items():
        dout = (DIMS_X if tag == "x" else DIMS_Q)[li][1]
        mtiles = (dout + 127) // 128
        bb = np.zeros((min(dout, 128), mtiles), np.float32)
        for m in range(mtiles):
            seg = b[m * 128:min((m + 1) * 128, dout)]
            bb[:len(seg), m] = seg
        barr[tag, li] = bb

    shared = {"sfreq": s, "sfreq2": s2,
              "ident3": np.eye(3, dtype=np.float32),
              "ident4": np.eye(4, dtype=np.float32)}
    for (tag, li), w in wmap.items():
        shared[f"w{tag}{li}"] = w
    for (tag, li), bb in barr.items():
        shared[f"b{tag}{li}"] = bb

    in_maps = []
    for c in range(N_CORES):
        cs = slice(c * S, (c + 1) * S)
        m = dict(shared)
        m["repx"] = np.ascontiguousarray(repx[:, cs])
        m["repq"] = np.ascontiguousarray(repq[:, cs])
        in_maps.append(m)
    return in_maps


def kernel(x, q, t, params_x, params_q, _trace=False):
    if "nc" not in _BUILT:
        _BUILT["nc"] = _build_program()
    nc = _BUILT["nc"]
    in_maps = _prep_host(x, q, t, params_x, params_q)
    res = run_bass_kernel_spmd(nc, in_maps, core_ids=list(range(N_CORES)),
                               trace=_trace)
    out_x = np.concatenate([res.results[c]["out_x"] for c in range(N_CORES)], axis=0)
    out_q = np.concatenate([res.results[c]["out_q"] for c in range(N_CORES)], axis=0)
    if _trace:
        kernel._last_exec_time_ns = res.exec_time_ns
        kernel._last_results = res
    return out_x, out_q
